# revision 4
# baseline (speedup 1.0000x reference)
"""GemmaAttention (B=2,S=2048,D=2048,H=8,KVH=1,HD=256) on 8 trn2 NeuronCores.

Sharding: 8 cores = 2 batches x 4 head-pairs (tensor parallel over the 8
query heads, data parallel over batch). Each core computes Q/K/V
projections + RoPE for its batch, causal attention for its 2 heads, and a
partial output projection. Host sums the 4 partial out-projections per
batch and reassembles the (transposed, causally-packed) attention output.

All matmuls run in bf16 with fp32 PSUM accumulation. Scores are produced
transposed ([k,128] x [q,512] tiles) so the exp'd tiles feed the ctx
matmul directly as stationary operands; row sums come from a ones-column
matmul, and the 1/sum normalization uses a PE-broadcast of the reciprocal
row.
"""

import sys

sys.path.insert(0, "/opt/trn_rl_repo")

import numpy as np
import ml_dtypes

import concourse.bacc as bacc
import concourse.tile as tile
import concourse.mybir as mybir
from concourse.bass_utils import run_bass_kernel_spmd

B, S, D = 2, 2048, 2048
H, KVH, HD = 8, 1, 256
ROPE_THETA = 10000.0
HH = HD // 2  # 128, rope half
NDK = D // 128  # 16 contraction chunks
NQC = S // 512  # 4 query chunks
NKB = S // 128  # 16 key blocks
SCALE = 1.0 / np.sqrt(HD)  # 0.0625

F32 = mybir.dt.float32
BF16 = mybir.dt.bfloat16
AF = mybir.ActivationFunctionType

_prog_cache = {}


def _build_program():
    if "nc" in _prog_cache:
        return _prog_cache["nc"]
    nc = bacc.Bacc("TRN2", target_bir_lowering=False, debug=False)

    # ---- DRAM I/O (per-core shards; same program on all 8 cores) ----
    xT = nc.dram_tensor("xT", [D, S], BF16, kind="ExternalInput")  # hidden[b].T
    wqT = nc.dram_tensor("wqT", [D, 2 * HD], BF16, kind="ExternalInput")
    wkT = nc.dram_tensor("wkT", [D, HD], BF16, kind="ExternalInput")
    wvT = nc.dram_tensor("wvT", [D, HD], BF16, kind="ExternalInput")
    woT = nc.dram_tensor("woT", [2 * HD, D], BF16, kind="ExternalInput")
    cosT = nc.dram_tensor("cosT", [HH, S], F32, kind="ExternalInput")
    sinT = nc.dram_tensor("sinT", [HH, S], F32, kind="ExternalInput")
    # packed transposed attention: [head, ki, qc, k=128, q=512]
    attn_pk = nc.dram_tensor("attn_pk", [2, NKB, NQC, 128, 512], F32,
                             kind="ExternalOutput")
    out_part = nc.dram_tensor("out_part", [S, D], F32, kind="ExternalOutput")

    with tile.TileContext(nc) as tc:
        with (
            tc.tile_pool(name="pers", bufs=1) as pers,
        ):
            ones_col = pers.tile([128, 1], BF16, tag="ones_col", name="ones_col")
            nc.vector.memset(ones_col[:], 1.0)
            ones_row = pers.tile([1, 128], F32, tag="ones_row", name="ones_row")
            nc.vector.memset(ones_row[:], 1.0)

            # persistent projection outputs (bf16)
            qt = {(h, hf): pers.tile([128, S], BF16, tag=f"qt{h}{hf}", name=f"qt{h}{hf}")
                  for h in range(2) for hf in range(2)}
            kt = {hf: pers.tile([128, S], BF16, tag=f"kt{hf}", name=f"kt{hf}") for hf in range(2)}
            vt = [pers.tile([128, HD], BF16, tag=f"vt{ki}", name=f"vt{ki}") for ki in range(NKB)]
            ctx_sb = {}  # (h, hf, qc) -> [128,512] bf16

            # ================= Phase P: projections + rope =================
            with (
                tc.tile_pool(name="px", bufs=1) as px,
                tc.tile_pool(name="pw", bufs=1) as pw,
                tc.tile_pool(name="ptrig", bufs=1) as ptrig,
                tc.tile_pool(name="ptmp", bufs=2) as ptmp,
                tc.tile_pool(name="ppsum", bufs=4, space="PSUM") as ppsum,
            ):
                xt = [px.tile([128, S], BF16, tag=f"xt{dk}", name=f"xt{dk}") for dk in range(NDK)]
                for dk in range(NDK):
                    nc.sync.dma_start(xt[dk][:], xT.ap()[dk * 128:(dk + 1) * 128, :])
                wq_sb = [pw.tile([128, 2 * HD], BF16, tag=f"wq{dk}", name=f"wq{dk}") for dk in range(NDK)]
                wk_sb = [pw.tile([128, HD], BF16, tag=f"wk{dk}", name=f"wk{dk}") for dk in range(NDK)]
                wv_sb = [pw.tile([128, HD], BF16, tag=f"wv{dk}", name=f"wv{dk}") for dk in range(NDK)]
                for dk in range(NDK):
                    sl = slice(dk * 128, (dk + 1) * 128)
                    nc.sync.dma_start(wq_sb[dk][:], wqT.ap()[sl, :])
                    nc.sync.dma_start(wk_sb[dk][:], wkT.ap()[sl, :])
                    nc.sync.dma_start(wv_sb[dk][:], wvT.ap()[sl, :])
                cos_sb = ptrig.tile([HH, S], F32, tag="cos", name="cos")
                sin_sb = ptrig.tile([HH, S], F32, tag="sin", name="sin")
                nc.sync.dma_start(cos_sb[:], cosT.ap())
                nc.sync.dma_start(sin_sb[:], sinT.ap())

                def rope_pair(dst0, dst1, ps0, ps1, sc):
                    ssl = slice(sc * 512, (sc + 1) * 512)
                    c = cos_sb[:, ssl]
                    s = sin_sb[:, ssl]
                    t0 = ptmp.tile([128, 512], F32, tag="t0", name="t0")
                    t1 = ptmp.tile([128, 512], F32, tag="t1", name="t1")
                    nc.vector.tensor_mul(t0[:], ps0[:], c)
                    nc.vector.tensor_mul(t1[:], ps1[:], s)
                    nc.vector.tensor_sub(dst0, t0[:], t1[:])
                    t2 = ptmp.tile([128, 512], F32, tag="t2", name="t2")
                    t3 = ptmp.tile([128, 512], F32, tag="t3", name="t3")
                    nc.vector.tensor_mul(t2[:], ps1[:], c)
                    nc.vector.tensor_mul(t3[:], ps0[:], s)
                    nc.vector.tensor_add(dst1, t2[:], t3[:])

                # Q/K (transposed layout [hd, s]) with rope
                qk_jobs = [("q", h, sc) for h in range(2) for sc in range(NQC)]
                qk_jobs += [("k", 0, sc) for sc in range(NQC)]
                for kind, h, sc in qk_jobs:
                    ssl = slice(sc * 512, (sc + 1) * 512)
                    pss = []
                    for hf in range(2):
                        ps = ppsum.tile([128, 512], F32, tag="proj", name="proj")
                        if kind == "q":
                            w = wq_sb
                            col = slice((2 * h + hf) * 128, (2 * h + hf + 1) * 128)
                        else:
                            w = wk_sb
                            col = slice(hf * 128, (hf + 1) * 128)
                        for dk in range(NDK):
                            nc.tensor.matmul(ps[:], w[dk][:, col], xt[dk][:, ssl],
                                             start=(dk == 0), stop=(dk == NDK - 1))
                        pss.append(ps)
                    if kind == "q":
                        rope_pair(qt[(h, 0)][:, ssl], qt[(h, 1)][:, ssl],
                                  pss[0], pss[1], sc)
                    else:
                        rope_pair(kt[0][:, ssl], kt[1][:, ssl], pss[0], pss[1], sc)

                # V natural layout [s, hd]
                for ki in range(NKB):
                    ps = ppsum.tile([128, HD], F32, tag="projv", name="projv")
                    ksl = slice(ki * 128, (ki + 1) * 128)
                    for dk in range(NDK):
                        nc.tensor.matmul(ps[:], xt[dk][:, ksl], wv_sb[dk][:],
                                         start=(dk == 0), stop=(dk == NDK - 1))
                    nc.scalar.copy(vt[ki][:], ps[:])

            # ================= Phase A: attention =================
            with (
                tc.tile_pool(name="pexp", bufs=20) as pexp,
                tc.tile_pool(name="pao", bufs=4) as pao,
                tc.tile_pool(name="pinv", bufs=2) as pinv,
                tc.tile_pool(name="psc", bufs=2, space="PSUM") as psc,
                tc.tile_pool(name="pctx", bufs=1, space="PSUM") as pctx,
                tc.tile_pool(name="psum2", bufs=2, space="PSUM") as psum2,
                tc.tile_pool(name="pinvb", bufs=1, space="PSUM") as pinvb,
            ):
                for h in range(2):
                    for qc in range(NQC):
                        qsl = slice(qc * 512, (qc + 1) * 512)
                        nki = 4 * qc + 4
                        sums = psum2.tile([1, 512], F32, tag="sums", name="sums")
                        cps = [pctx.tile([128, 512], F32, tag=f"ctx{hf}", name=f"ctx{hf}")
                               for hf in range(2)]
                        exp_tiles = []
                        for ki in range(nki):
                            ksl = slice(ki * 128, (ki + 1) * 128)
                            sp = psc.tile([128, 512], F32, tag="sc", name="sc")
                            nc.tensor.matmul(sp[:], kt[0][:, ksl], qt[(h, 0)][:, qsl],
                                             start=True, stop=False)
                            nc.tensor.matmul(sp[:], kt[1][:, ksl], qt[(h, 1)][:, qsl],
                                             start=False, stop=True)
                            et = pexp.tile([128, 512], BF16, tag="exp", name="exp")
                            nc.scalar.activation(et[:], sp[:], AF.Exp, scale=SCALE)
                            if ki >= 4 * qc:
                                # keep where (512*qc + f) - (128*ki + p) >= 0
                                nc.gpsimd.affine_select(
                                    out=et[:], in_=et[:],
                                    compare_op=mybir.AluOpType.is_ge,
                                    fill=0.0, base=512 * qc - 128 * ki,
                                    pattern=[[1, 512]], channel_multiplier=-1,
                                )
                            first, last = ki == 0, ki == nki - 1
                            nc.tensor.matmul(sums[:], ones_col[:], et[:],
                                             start=first, stop=last)
                            for hf in range(2):
                                hsl = slice(hf * 128, (hf + 1) * 128)
                                nc.tensor.matmul(cps[hf][:], vt[ki][:, hsl], et[:],
                                                 start=first, stop=last)
                            exp_tiles.append((ki, et))
                        inv = pinv.tile([1, 512], F32, tag="inv", name="inv")
                        nc.vector.reciprocal(inv[:], sums[:])
                        invb = pinvb.tile([128, 512], F32, tag="invb", name="invb")
                        nc.tensor.matmul(invb[:], ones_row[:], inv[:],
                                         start=True, stop=True)
                        invs = pinv.tile([128, 512], F32, tag="invs", name="invs")
                        nc.scalar.copy(invs[:], invb[:])
                        for ki, et in exp_tiles:
                            ao = pao.tile([128, 512], F32, tag="ao", name="ao")
                            nc.vector.tensor_mul(ao[:], et[:], invs[:])
                            vo = max(0, 128 * ki - 512 * qc)
                            nc.sync.dma_start(
                                attn_pk.ap()[h, ki, qc, :, vo:], ao[:, vo:])
                        for hf in range(2):
                            ct = pers.tile([128, 512], BF16, tag=f"ct{h}{hf}{qc}", name=f"ct{h}{hf}{qc}")
                            nc.vector.tensor_mul(ct[:], cps[hf][:], invs[:])
                            ctx_sb[(h, hf, qc)] = ct

            # ================= Phase O: output projection =================
            with (
                tc.tile_pool(name="po", bufs=4, space="PSUM") as po,
                tc.tile_pool(name="pos", bufs=4) as pos,
                tc.tile_pool(name="pwo", bufs=1) as pwo,
            ):
                wo_sb = [pwo.tile([128, D], BF16, tag=f"wo{j}", name=f"wo{j}")
                         for j in range(4)]
                for j in range(4):
                    nc.sync.dma_start(wo_sb[j][:], woT.ap()[j * 128:(j + 1) * 128, :])
                for qb in range(16):
                    qc, co = qb // 4, (qb % 4) * 128
                    for dc in range(4):
                        ps = po.tile([128, 512], F32, tag="o", name="o")
                        for idx in range(4):
                            h, hf = idx // 2, idx % 2
                            nc.tensor.matmul(
                                ps[:], ctx_sb[(h, hf, qc)][:, co:co + 128],
                                wo_sb[idx][:, dc * 512:(dc + 1) * 512],
                                start=(idx == 0), stop=(idx == 3))
                        ot = pos.tile([128, 512], F32, tag="ot", name="ot")
                        nc.scalar.copy(ot[:], ps[:])
                        nc.sync.dma_start(
                            out_part.ap()[qb * 128:(qb + 1) * 128,
                                          dc * 512:(dc + 1) * 512], ot[:])

    nc.compile()
    _prog_cache["nc"] = nc
    return nc


def _make_in_maps(hidden_states, position_ids, Wq, Wk, Wv, Wo):
    bf = ml_dtypes.bfloat16
    inv_freq = (1.0 / (ROPE_THETA ** (np.arange(0, HD, 2, dtype=np.float32) / HD)))
    in_maps = []
    per_batch = {}
    for b in range(B):
        pos = np.asarray(position_ids[b], dtype=np.float32)
        freqs = pos[:, None] * inv_freq[None, :]  # [S, HH]
        per_batch[b] = {
            "xT": np.ascontiguousarray(
                np.asarray(hidden_states[b]).T.astype(bf)),
            "wkT": np.ascontiguousarray(np.asarray(Wk).T.astype(bf)),
            "wvT": np.ascontiguousarray(np.asarray(Wv).T.astype(bf)),
            "cosT": np.ascontiguousarray(np.cos(freqs).T.astype(np.float32)),
            "sinT": np.ascontiguousarray(np.sin(freqs).T.astype(np.float32)),
        }
    for c in range(8):
        b, hp = c // 4, c % 4
        h0 = 2 * hp
        m = dict(per_batch[b])
        m["wqT"] = np.ascontiguousarray(
            np.asarray(Wq)[h0 * HD:(h0 + 2) * HD, :].T.astype(bf))
        m["woT"] = np.ascontiguousarray(
            np.asarray(Wo)[:, h0 * HD:(h0 + 2) * HD].T.astype(bf))
        in_maps.append(m)
    return in_maps


def _run(hidden_states, attention_mask, position_ids, Wq, Wk, Wv, Wo, **rk):
    nc = _build_program()
    in_maps = _make_in_maps(hidden_states, position_ids, Wq, Wk, Wv, Wo)
    res = run_bass_kernel_spmd(nc, in_maps, core_ids=list(range(8)), **rk)

    out = np.zeros((B, S, D), dtype=np.float32)
    attn = np.zeros((B, H, S, S), dtype=np.float32)
    for c in range(8):
        b, hp = c // 4, c % 4
        r = res.results[c]
        out[b] += r["out_part"]
        pk = r["attn_pk"]  # [2, NKB, NQC, 128, 512]
        for hh in range(2):
            g = 2 * hp + hh
            for ki in range(NKB):
                qcm = ki // 4
                blk = pk[hh, ki, qcm:]  # [nv, 128, 512]
                attn[b, g, 512 * qcm:, ki * 128:(ki + 1) * 128] = (
                    blk.transpose(0, 2, 1).reshape(-1, 128))
    return (out, attn), res


def kernel(hidden_states, attention_mask, position_ids, Wq, Wk, Wv, Wo):
    (out, attn), _ = _run(hidden_states, attention_mask, position_ids,
                          Wq, Wk, Wv, Wo)
    return out, attn
